# revision 16
# baseline (speedup 1.0000x reference)
"""DimensionalConsistencyLoss on 8 Trainium2 NeuronCores.

The loss touches only gathered rows of the [100000, 512] f32 table: 8192
pos/neg constraints read one row each (sparsity term + target element), 2048
neu constraints read one element. Everything is fetched with row gathers.

Per core (1/8 of the constraints, dealt by the host):
  - 4x `dma_gather` (SWDGE custom gather, ~1us fixed + 0.34ns/descriptor)
    fetch all rows in one instruction per vocab range. int16 gather indices
    only reach 32768 rows, so ids are bucketed into 4 ranges and the
    per-bucket `in_ap` base offset covers the rest. Buckets are padded to a
    static capacity with row-0 dummies; a zero coefficient kills their
    contribution.
  - ACT: per 512-wide column tile, activation(Abs, accum_out) produces the
    row |.| sums in one pass.
  - DVE: per tile, scalar_tensor_tensor((ramp == dim_p) * row, accum_out)
    extracts the target element t in one pass.
  - Per-slot coefficient arrays (host-built) unify pos/neg/neu/pad:
        L = w*(Q*|t| + R) + P*|t| + C*rowsum,   w = (S*t >= 0)
  - ones-matmul reduces the [128, COLS] loss matrix to one scalar.

Host sums 8 partial scalars and applies the final scale.
"""

import numpy as np

import concourse.bacc as bacc
import concourse.bass as bass
import concourse.mybir as mybir
from concourse.bass_utils import run_bass_kernel_spmd
from concourse.library_config import mlp

P = 128
VOCAB = 100000
DIM = 512
N_POS = 4096
N_NEG = 4096
N_NEU = 2048
N_ALL = N_POS + N_NEG + N_NEU
N_CORES = 8

RANGE = 32768                      # int16 index reach (rows)
BASES = [0, 32768, 65536, 98304]
WIDTHS = [32768, 32768, 32768, VOCAB - 98304]
CAPS = [512, 512, 512, 128]        # per-core per-bucket slot capacity
NB = len(CAPS)
SLOTS = sum(CAPS)                  # 1664
COLS = SLOTS // P                  # 13
COL_OFF = [0, 4, 8, 12]            # column offset of each bucket
GORDER = [3, 0, 1, 2]              # gather issue order: small bucket first
I16_OFF = [c // 16 for c in np.cumsum([0] + CAPS[:-1])]  # [0, 32, 64, 96]

CONSISTENCY_WEIGHT = 0.5
SPARSITY_WEIGHT = 0.1
C_SP = SPARSITY_WEIGHT / (DIM - 1)

# coefs tensor layout (f32, [128, CW_TOT]): ramp | dims | S | Pp | Q | R | Cc | ones
CW_RAMP = DIM
C_DIMS = CW_RAMP
C_S = C_DIMS + COLS
C_PP = C_S + COLS
C_Q = C_PP + COLS
C_R = C_Q + COLS
C_CC = C_R + COLS
C_ONE = C_CC + COLS
CW_TOT = C_ONE + 1

F32 = mybir.dt.float32
I16 = mybir.dt.int16
AX = mybir.AxisListType.X
OP = mybir.AluOpType
AF = mybir.ActivationFunctionType

_nc_cache = None


def _build_program():
    global _nc_cache
    if _nc_cache is not None:
        return _nc_cache

    nc = bacc.Bacc(
        "TRN2", target_bir_lowering=False, debug=False, num_devices=N_CORES,
        num_swdge_queues=4,
    )
    emb = nc.dram_tensor("emb", [VOCAB, DIM], F32, kind="ExternalInput")
    idx_d = nc.dram_tensor("idx16", [P, SLOTS // 16], I16, kind="ExternalInput")
    coef_d = nc.dram_tensor("coefs", [P, CW_TOT], F32, kind="ExternalInput")
    out_d = nc.dram_tensor("out", [1, 1], F32, kind="ExternalOutput")

    from contextlib import ExitStack

    with ExitStack() as ctx:
        block = ctx.enter_context(nc.Block())
        sb = lambda name, shape, dt=F32: ctx.enter_context(
            nc.sbuf_tensor(name, shape, dt)
        )
        idx_sb = sb("idx_sb", [P, SLOTS // 16], I16)
        coef_sb = sb("coef_sb", [P, CW_TOT])
        rows = sb("rows", [P, COLS, DIM])
        s_act = sb("s_act", [P, COLS, DIM])
        s_dve = sb("s_dve", [P, COLS, DIM])
        rowsum = sb("rowsum", [P, COLS])
        tcol = sb("tcol", [P, COLS])
        a13 = sb("a13", [P, COLS])
        u13 = sb("u13", [P, COLS])
        w13 = sb("w13", [P, COLS])
        x1 = sb("x1", [P, COLS])
        x2 = sb("x2", [P, COLS])
        x3 = sb("x3", [P, COLS])
        m13 = sb("m13", [P, COLS])
        Lc = sb("Lc", [P, 1])
        res = sb("res", [1, 1])
        acc = ctx.enter_context(nc.psum_tensor([1, 1], F32))
        sem = lambda name: ctx.enter_context(nc.semaphore(name))
        io, io_i, io2 = sem("io"), sem("io_i"), sem("io2")
        gs = [sem(f"gs{b}") for b in range(NB)]
        dve_x, act_s, dve_f = sem("dve_x"), sem("act_s"), sem("dve_f")
        pe_s, cp_s = sem("pe_s"), sem("cp_s")
        chain_len = {}
        ramp = coef_sb[:, 0:CW_RAMP]
        ones = coef_sb[:, C_ONE : C_ONE + 1]

        @block.sync
        def _(sync: bass.BassEngine):
            sync.dma_start(coef_sb[:, :], coef_d[:, :]).then_inc(io, 16)
            sync.wait_ge(cp_s, 1)
            sync.dma_start(out_d[:, :], res[:, :]).then_inc(io2, 16)
            sync.wait_ge(io2, 16)

        @block.gpsimd
        def _(gpsimd: bass.BassGpSimd):
            gpsimd.load_library(mlp)
            gpsimd.dma_start(idx_sb[:, :], idx_d[:, :]).then_inc(io_i, 16)
            gpsimd.wait_ge(io_i, 16)
            # Distinct queue_num -> each gather runs on its own Q7 cpu pair.
            # The first gather blocks the next dispatch until its pair is
            # done, so issue the small bucket first.
            for q, b in enumerate(GORDER):
                gpsimd.dma_gather(
                    rows[:, COL_OFF[b] : COL_OFF[b] + CAPS[b] // P, :],
                    emb[BASES[b] : BASES[b] + WIDTHS[b], :],
                    idx_sb[:, I16_OFF[b] : I16_OFF[b] + CAPS[b] // 16],
                    CAPS[b],
                    CAPS[b],
                    DIM,
                    queue_num=q,
                ).then_inc(gs[b], 16)

        @block.scalar
        def _(scalar: bass.BassEngine):
            for b in GORDER:
                scalar.wait_ge(gs[b], 16)
                for j in range(COL_OFF[b], COL_OFF[b] + CAPS[b] // P):
                    nc.scalar.activation(
                        s_act[:, j, :], rows[:, j, :], AF.Abs,
                        accum_out=rowsum[:, j : j + 1],
                    ).then_inc(act_s, 1)
            scalar.wait_ge(pe_s, 1)
            nc.scalar.copy(res[:, :], acc[:, :]).then_inc(cp_s, 1)

        @block.vector
        def _(vector: bass.BassEngine):
            vector.wait_ge(io, 16)
            for b in GORDER:
                vector.wait_ge(gs[b], 16)
                for j in range(COL_OFF[b], COL_OFF[b] + CAPS[b] // P):
                    nc.vector.scalar_tensor_tensor(
                        out=s_dve[:, j, :],
                        in0=ramp,
                        scalar=coef_sb[:, C_DIMS + j : C_DIMS + j + 1],
                        in1=rows[:, j, :],
                        op0=OP.is_equal,
                        op1=OP.mult,
                        accum_out=tcol[:, j : j + 1],
                    ).then_inc(dve_x, 1)
            # accum_out writes land late; drain our own pipeline before reads
            vector.wait_ge(dve_x, COLS)
            # Same-engine RAW needs explicit sems (deep DVE pipeline).
            # dve_f counts completions; wait on the latest producer.
            # L = w*(Q*a + R) + Pp*a + Cc*rowsum,  w = (t*S>=0), a = |t|
            n = 0

            def step(ins, wait=None):
                nonlocal n
                if wait is not None:
                    vector.wait_ge(dve_f, wait)
                ins().then_inc(dve_f, 1)
                n += 1
                return n

            tS = coef_sb[:, C_S : C_S + COLS]
            i_u = step(lambda: nc.vector.tensor_tensor(
                out=u13[:, :], in0=tcol[:, :], in1=tS, op=OP.mult))
            i_w = step(lambda: nc.vector.tensor_scalar(
                out=w13[:, :], in0=u13[:, :], scalar1=0.0, scalar2=None,
                op0=OP.is_ge), wait=i_u)
            i_m = step(lambda: nc.vector.tensor_scalar(
                out=m13[:, :], in0=w13[:, :], scalar1=2.0, scalar2=-1.0,
                op0=OP.mult, op1=OP.add), wait=i_w)
            i_a = step(lambda: nc.vector.tensor_tensor(
                out=a13[:, :], in0=u13[:, :], in1=m13[:, :], op=OP.mult),
                wait=i_m)
            i1 = step(lambda: nc.vector.tensor_tensor(
                out=x1[:, :], in0=a13[:, :], in1=coef_sb[:, C_Q : C_Q + COLS],
                op=OP.mult), wait=i_a)
            i2 = step(lambda: nc.vector.tensor_tensor(
                out=x2[:, :], in0=a13[:, :], in1=coef_sb[:, C_PP : C_PP + COLS],
                op=OP.mult))
            vector.wait_ge(act_s, COLS)
            i3 = step(lambda: nc.vector.tensor_tensor(
                out=x3[:, :], in0=rowsum[:, :], in1=coef_sb[:, C_CC : C_CC + COLS],
                op=OP.mult))
            i4 = step(lambda: nc.vector.tensor_tensor(
                out=x1[:, :], in0=x1[:, :], in1=coef_sb[:, C_R : C_R + COLS],
                op=OP.add), wait=i1)
            i5 = step(lambda: nc.vector.tensor_tensor(
                out=x1[:, :], in0=x1[:, :], in1=w13[:, :], op=OP.mult), wait=i4)
            i6 = step(lambda: nc.vector.tensor_tensor(
                out=x1[:, :], in0=x1[:, :], in1=x2[:, :], op=OP.add),
                wait=max(i5, i2))
            i7 = step(lambda: nc.vector.tensor_tensor(
                out=x1[:, :], in0=x1[:, :], in1=x3[:, :], op=OP.add),
                wait=max(i6, i3))
            i8 = step(lambda: nc.vector.reduce_sum(
                Lc[:, :], x1[:, :], axis=AX), wait=i7)
            chain_len["n"] = i8

        @block.tensor
        def _(tensor: bass.BassEngine):
            tensor.wait_ge(io, 16)
            tensor.wait_ge(dve_f, chain_len["n"])
            nc.tensor.matmul(
                acc[:, :], lhsT=ones, rhs=Lc[:, :], start=True, stop=True
            ).then_inc(pe_s, 1)

    nc.compile()
    _nc_cache = nc
    return nc


def _deal(pos_ids, pos_dims, neg_ids, neg_dims, neu_ids, neu_dims):
    """Deal all constraints into per-core, per-bucket slot tables.

    Returns per-core (idx16 [16, SLOTS//16] int16, coefs [128, CW_TOT] f32).
    """
    ids = np.concatenate([pos_ids, neg_ids, neu_ids]).astype(np.int64)
    dims = np.concatenate([pos_dims, neg_dims, neu_dims]).astype(np.int64)
    cls = np.concatenate([
        np.zeros(len(pos_ids), np.int64),
        np.ones(len(neg_ids), np.int64),
        np.full(len(neu_ids), 2, np.int64),
    ])
    bucket = ids // RANGE  # 98304+ lands in 3

    idx16 = [np.zeros((P, SLOTS // 16), np.int16) for _ in range(N_CORES)]
    coefs = [np.zeros((P, CW_TOT), np.float32) for _ in range(N_CORES)]
    for c in range(N_CORES):
        coefs[c][:, 0:CW_RAMP] = np.arange(DIM, dtype=np.float32)[None, :]
        coefs[c][:, C_S : C_S + COLS] = 1.0
        coefs[c][:, C_ONE] = 1.0

    for b in range(NB):
        sel = np.where(bucket == b)[0]
        for m, g in enumerate(sel):
            c, j = m % N_CORES, m // N_CORES
            if j >= CAPS[b]:
                raise RuntimeError(
                    f"bucket {b} overflow on core {c}: {len(sel)} ids for "
                    f"8x{CAPS[b]} slots"
                )
            idx16[c][j % 16 :: 16, I16_OFF[b] + j // 16] = ids[g] - BASES[b]
            p, col = j % P, COL_OFF[b] + j // P
            k = cls[g]
            coefs[c][p, C_DIMS + col] = dims[g]
            if k == 0:
                coefs[c][p, C_S + col] = -1.0
            if k == 2:
                coefs[c][p, C_PP + col] = 2.0
            else:
                coefs[c][p, C_PP + col] = -SPARSITY_WEIGHT - C_SP
                coefs[c][p, C_Q + col] = 1.0 + SPARSITY_WEIGHT
                coefs[c][p, C_R + col] = SPARSITY_WEIGHT
                coefs[c][p, C_CC + col] = C_SP
    # note: pad slots keep idx 0 / all-zero coefficients -> contribute 0
    return idx16, coefs


def _make_in_maps(emb, pos_ids, pos_dims, neg_ids, neg_dims, neu_ids, neu_dims):
    idx16, coefs = _deal(pos_ids, pos_dims, neg_ids, neg_dims, neu_ids, neu_dims)
    return [
        {"emb": emb, "idx16": idx16[c], "coefs": coefs[c]}
        for c in range(N_CORES)
    ]


def kernel(**inputs):
    emb = np.ascontiguousarray(np.asarray(inputs["embeddings"], dtype=np.float32))
    ids = {
        k: np.asarray(inputs[k]).astype(np.int64)
        for k in ("pos_ids", "pos_dims", "neg_ids", "neg_dims", "neu_ids", "neu_dims")
    }
    nc = _build_program()
    in_maps = _make_in_maps(
        emb, ids["pos_ids"], ids["pos_dims"], ids["neg_ids"], ids["neg_dims"],
        ids["neu_ids"], ids["neu_dims"],
    )
    res = run_bass_kernel_spmd(nc, in_maps, list(range(N_CORES)))
    total = sum(float(r["out"][0, 0]) for r in res.results)
    val = total * CONSISTENCY_WEIGHT / N_ALL
    return np.asarray(val, dtype=np.float32)


# revision 17
# speedup vs baseline: 1.3802x; 1.3802x over previous
"""DimensionalConsistencyLoss on 8 Trainium2 NeuronCores.

The loss touches only gathered rows of the [100000, 512] f32 table: 8192
pos/neg constraints read one row each (sparsity term + target element), 2048
neu constraints read one element. Everything is fetched with row gathers.

Per core (1/8 of the constraints = 1280 slots = 10 columns of 128, dealt by
the host):
  - 10x indirect-DMA row gathers (one [128,512] tile per column; the SWDGE
    Q7 feeds descriptors faster than the 16 SDMA engines drain them, and
    unlike dma_gather this needs no ucode-library load, which costs ~10us).
  - ACT: per tile, activation(Abs, accum_out) -> row |.| sums in one pass.
  - DVE: per tile, scalar_tensor_tensor((ramp == dim_p) * row, accum_out)
    extracts the target element t in one pass.
  - Per-slot coefficient arrays (host-built) unify pos/neg/neu:
        L = w*(Q*|t| + R) + P*|t| + C*rowsum,   w = (S*t >= 0)
  - ones-matmul reduces the [128, COLS] loss matrix to one scalar.

Host sums 8 partial scalars and applies the final scale.
"""

import numpy as np

import concourse.bacc as bacc
import concourse.bass as bass
import concourse.mybir as mybir
from concourse.bass_utils import run_bass_kernel_spmd

P = 128
VOCAB = 100000
DIM = 512
N_POS = 4096
N_NEG = 4096
N_NEU = 2048
N_ALL = N_POS + N_NEG + N_NEU
N_CORES = 8

SLOTS = N_ALL // N_CORES           # 1280
COLS = SLOTS // P                  # 10

CONSISTENCY_WEIGHT = 0.5
SPARSITY_WEIGHT = 0.1
C_SP = SPARSITY_WEIGHT / (DIM - 1)

# coefs tensor layout (f32, [128, CW_TOT]): ramp | dims | S | Pp | Q | R | Cc | ones
CW_RAMP = DIM
C_DIMS = CW_RAMP
C_S = C_DIMS + COLS
C_PP = C_S + COLS
C_Q = C_PP + COLS
C_R = C_Q + COLS
C_CC = C_R + COLS
C_ONE = C_CC + COLS
CW_TOT = C_ONE + 1

F32 = mybir.dt.float32
I32 = mybir.dt.int32
AX = mybir.AxisListType.X
OP = mybir.AluOpType
AF = mybir.ActivationFunctionType

_nc_cache = None


def _build_program():
    global _nc_cache
    if _nc_cache is not None:
        return _nc_cache

    nc = bacc.Bacc(
        "TRN2", target_bir_lowering=False, debug=False, num_devices=N_CORES
    )
    emb = nc.dram_tensor("emb", [VOCAB, DIM], F32, kind="ExternalInput")
    idx_d = nc.dram_tensor("idx32", [P, COLS], I32, kind="ExternalInput")
    coef_d = nc.dram_tensor("coefs", [P, CW_TOT], F32, kind="ExternalInput")
    out_d = nc.dram_tensor("out", [1, 1], F32, kind="ExternalOutput")

    from contextlib import ExitStack

    with ExitStack() as ctx:
        block = ctx.enter_context(nc.Block())
        sb = lambda name, shape, dt=F32: ctx.enter_context(
            nc.sbuf_tensor(name, shape, dt)
        )
        idx_sb = sb("idx_sb", [P, COLS], I32)
        coef_sb = sb("coef_sb", [P, CW_TOT])
        rows = sb("rows", [P, COLS, DIM])
        s_act = sb("s_act", [P, COLS, DIM])
        s_dve = sb("s_dve", [P, COLS, DIM])
        rowsum = sb("rowsum", [P, COLS])
        tcol = sb("tcol", [P, COLS])
        a13 = sb("a13", [P, COLS])
        u13 = sb("u13", [P, COLS])
        w13 = sb("w13", [P, COLS])
        x1 = sb("x1", [P, COLS])
        x2 = sb("x2", [P, COLS])
        x3 = sb("x3", [P, COLS])
        m13 = sb("m13", [P, COLS])
        Lc = sb("Lc", [P, 1])
        res = sb("res", [1, 1])
        acc = ctx.enter_context(nc.psum_tensor([1, 1], F32))
        sem = lambda name: ctx.enter_context(nc.semaphore(name))
        io, io_i, io2 = sem("io"), sem("io_i"), sem("io2")
        gsem = sem("gsem")
        dve_x, act_s, dve_f = sem("dve_x"), sem("act_s"), sem("dve_f")
        pe_s, cp_s = sem("pe_s"), sem("cp_s")
        chain_len = {}
        ramp = coef_sb[:, 0:CW_RAMP]
        ones = coef_sb[:, C_ONE : C_ONE + 1]

        @block.sync
        def _(sync: bass.BassEngine):
            sync.dma_start(idx_sb[:, :], idx_d[:, :]).then_inc(io_i, 16)
            sync.dma_start(coef_sb[:, :], coef_d[:, :]).then_inc(io, 16)
            sync.wait_ge(cp_s, 1)
            sync.dma_start(out_d[:, :], res[:, :]).then_inc(io2, 16)
            sync.wait_ge(io2, 16)

        @block.gpsimd
        def _(gpsimd: bass.BassGpSimd):
            gpsimd.wait_ge(io_i, 16)
            for j in range(COLS):
                gpsimd.indirect_dma_start(
                    out=rows[:, j, :],
                    out_offset=None,
                    in_=emb[:, :],
                    in_offset=bass.IndirectOffsetOnAxis(
                        ap=idx_sb[:, j : j + 1], axis=0
                    ),
                ).then_inc(gsem, 16)

        @block.scalar
        def _(scalar: bass.BassEngine):
            for j in range(COLS):
                scalar.wait_ge(gsem, 16 * (j + 1))
                nc.scalar.activation(
                    s_act[:, j, :], rows[:, j, :], AF.Abs,
                    accum_out=rowsum[:, j : j + 1],
                ).then_inc(act_s, 1)
            scalar.wait_ge(pe_s, 1)
            nc.scalar.copy(res[:, :], acc[:, :]).then_inc(cp_s, 1)

        @block.vector
        def _(vector: bass.BassEngine):
            vector.wait_ge(io, 16)
            for j in range(COLS):
                vector.wait_ge(gsem, 16 * (j + 1))
                nc.vector.scalar_tensor_tensor(
                    out=s_dve[:, j, :],
                    in0=ramp,
                    scalar=coef_sb[:, C_DIMS + j : C_DIMS + j + 1],
                    in1=rows[:, j, :],
                    op0=OP.is_equal,
                    op1=OP.mult,
                    accum_out=tcol[:, j : j + 1],
                ).then_inc(dve_x, 1)
            # accum_out writes land late; drain our own pipeline before reads
            vector.wait_ge(dve_x, COLS)
            # Same-engine RAW needs explicit sems (deep DVE pipeline).
            # dve_f counts completions; wait on the latest producer.
            # L = w*(Q*a + R) + Pp*a + Cc*rowsum,  w = (t*S>=0), a = |t|
            n = 0

            def step(ins, wait=None):
                nonlocal n
                if wait is not None:
                    vector.wait_ge(dve_f, wait)
                ins().then_inc(dve_f, 1)
                n += 1
                return n

            tS = coef_sb[:, C_S : C_S + COLS]
            i_u = step(lambda: nc.vector.tensor_tensor(
                out=u13[:, :], in0=tcol[:, :], in1=tS, op=OP.mult))
            i_w = step(lambda: nc.vector.tensor_scalar(
                out=w13[:, :], in0=u13[:, :], scalar1=0.0, scalar2=None,
                op0=OP.is_ge), wait=i_u)
            i_m = step(lambda: nc.vector.tensor_scalar(
                out=m13[:, :], in0=w13[:, :], scalar1=2.0, scalar2=-1.0,
                op0=OP.mult, op1=OP.add), wait=i_w)
            i_a = step(lambda: nc.vector.tensor_tensor(
                out=a13[:, :], in0=u13[:, :], in1=m13[:, :], op=OP.mult),
                wait=i_m)
            i1 = step(lambda: nc.vector.tensor_tensor(
                out=x1[:, :], in0=a13[:, :], in1=coef_sb[:, C_Q : C_Q + COLS],
                op=OP.mult), wait=i_a)
            i2 = step(lambda: nc.vector.tensor_tensor(
                out=x2[:, :], in0=a13[:, :], in1=coef_sb[:, C_PP : C_PP + COLS],
                op=OP.mult))
            vector.wait_ge(act_s, COLS)
            i3 = step(lambda: nc.vector.tensor_tensor(
                out=x3[:, :], in0=rowsum[:, :], in1=coef_sb[:, C_CC : C_CC + COLS],
                op=OP.mult))
            i4 = step(lambda: nc.vector.tensor_tensor(
                out=x1[:, :], in0=x1[:, :], in1=coef_sb[:, C_R : C_R + COLS],
                op=OP.add), wait=i1)
            i5 = step(lambda: nc.vector.tensor_tensor(
                out=x1[:, :], in0=x1[:, :], in1=w13[:, :], op=OP.mult), wait=i4)
            i6 = step(lambda: nc.vector.tensor_tensor(
                out=x1[:, :], in0=x1[:, :], in1=x2[:, :], op=OP.add),
                wait=max(i5, i2))
            i7 = step(lambda: nc.vector.tensor_tensor(
                out=x1[:, :], in0=x1[:, :], in1=x3[:, :], op=OP.add),
                wait=max(i6, i3))
            i8 = step(lambda: nc.vector.reduce_sum(
                Lc[:, :], x1[:, :], axis=AX), wait=i7)
            chain_len["n"] = i8

        @block.tensor
        def _(tensor: bass.BassEngine):
            tensor.wait_ge(io, 16)
            tensor.wait_ge(dve_f, chain_len["n"])
            nc.tensor.matmul(
                acc[:, :], lhsT=ones, rhs=Lc[:, :], start=True, stop=True
            ).then_inc(pe_s, 1)

    nc.compile()
    _nc_cache = nc
    return nc


def _deal(pos_ids, pos_dims, neg_ids, neg_dims, neu_ids, neu_dims):
    """Deal all constraints into per-core slot tables (slot j of core c =
    constraint c + 8*j of the concatenated list).

    Returns per-core (idx32 [128, COLS] int32, coefs [128, CW_TOT] f32).
    """
    ids = np.concatenate([pos_ids, neg_ids, neu_ids]).astype(np.int64)
    dims = np.concatenate([pos_dims, neg_dims, neu_dims]).astype(np.int64)
    cls = np.concatenate([
        np.zeros(len(pos_ids), np.int64),
        np.ones(len(neg_ids), np.int64),
        np.full(len(neu_ids), 2, np.int64),
    ])

    idx32 = []
    coefs = []
    for c in range(N_CORES):
        g = np.arange(SLOTS) * N_CORES + c  # this core's constraints
        cid, cdim, ccls = ids[g], dims[g], cls[g]
        # slot j -> (p = j%128, col = j//128)
        ix = np.ascontiguousarray(
            cid.reshape(COLS, P).T.astype(np.int32))  # [128, COLS]
        cf = np.zeros((P, CW_TOT), np.float32)
        cf[:, 0:CW_RAMP] = np.arange(DIM, dtype=np.float32)[None, :]
        cf[:, C_ONE] = 1.0
        dm = cdim.reshape(COLS, P).T
        kl = ccls.reshape(COLS, P).T
        cf[:, C_DIMS : C_DIMS + COLS] = dm
        cf[:, C_S : C_S + COLS] = np.where(kl == 0, -1.0, 1.0)
        pn = kl != 2
        cf[:, C_PP : C_PP + COLS] = np.where(
            pn, -SPARSITY_WEIGHT - C_SP, 2.0)
        cf[:, C_Q : C_Q + COLS] = np.where(pn, 1.0 + SPARSITY_WEIGHT, 0.0)
        cf[:, C_R : C_R + COLS] = np.where(pn, SPARSITY_WEIGHT, 0.0)
        cf[:, C_CC : C_CC + COLS] = np.where(pn, C_SP, 0.0)
        idx32.append(ix)
        coefs.append(cf)
    return idx32, coefs


def _make_in_maps(emb, pos_ids, pos_dims, neg_ids, neg_dims, neu_ids, neu_dims):
    idx32, coefs = _deal(pos_ids, pos_dims, neg_ids, neg_dims, neu_ids, neu_dims)
    return [
        {"emb": emb, "idx32": idx32[c], "coefs": coefs[c]}
        for c in range(N_CORES)
    ]


def kernel(**inputs):
    emb = np.ascontiguousarray(np.asarray(inputs["embeddings"], dtype=np.float32))
    ids = {
        k: np.asarray(inputs[k]).astype(np.int64)
        for k in ("pos_ids", "pos_dims", "neg_ids", "neg_dims", "neu_ids", "neu_dims")
    }
    nc = _build_program()
    in_maps = _make_in_maps(
        emb, ids["pos_ids"], ids["pos_dims"], ids["neg_ids"], ids["neg_dims"],
        ids["neu_ids"], ids["neu_dims"],
    )
    res = run_bass_kernel_spmd(nc, in_maps, list(range(N_CORES)))
    total = sum(float(r["out"][0, 0]) for r in res.results)
    val = total * CONSISTENCY_WEIGHT / N_ALL
    return np.asarray(val, dtype=np.float32)


# revision 18
# speedup vs baseline: 1.3848x; 1.0033x over previous
"""DimensionalConsistencyLoss on 8 Trainium2 NeuronCores.

The loss touches only gathered rows of the [100000, 512] f32 table: 8192
pos/neg constraints read one row each (sparsity term + target element), 2048
neu constraints read one element. Everything is fetched with row gathers.

Per core (1/8 of the constraints = 1280 slots = 10 columns of 128, dealt by
the host):
  - 10x indirect-DMA row gathers (one [128,512] tile per column; the SWDGE
    Q7 feeds descriptors faster than the 16 SDMA engines drain them, and
    unlike dma_gather this needs no ucode-library load, which costs ~10us).
  - ACT: per tile, activation(Abs, accum_out) -> row |.| sums in one pass.
  - DVE: per tile, scalar_tensor_tensor((ramp == dim_p) * row, accum_out)
    extracts the target element t in one pass.
  - Per-slot coefficient arrays (host-built) unify pos/neg/neu:
        L = w*(Q*|t| + R) + P*|t| + C*rowsum,   w = (S*t >= 0)
  - ones-matmul reduces the [128, COLS] loss matrix to one scalar.

Host sums 8 partial scalars and applies the final scale.
"""

import numpy as np

import concourse.bacc as bacc
import concourse.bass as bass
import concourse.mybir as mybir
from concourse.bass_utils import run_bass_kernel_spmd

P = 128
VOCAB = 100000
DIM = 512
N_POS = 4096
N_NEG = 4096
N_NEU = 2048
N_ALL = N_POS + N_NEG + N_NEU
N_CORES = 8

SLOTS = N_ALL // N_CORES           # 1280
COLS = SLOTS // P                  # 10

CONSISTENCY_WEIGHT = 0.5
SPARSITY_WEIGHT = 0.1
C_SP = SPARSITY_WEIGHT / (DIM - 1)

# coefs tensor layout (f32, [128, CW_TOT]): ramp | dims | S | Pp | Q | R | Cc | ones
CW_RAMP = DIM
C_DIMS = CW_RAMP
C_S = C_DIMS + COLS
C_PP = C_S + COLS
C_Q = C_PP + COLS
C_R = C_Q + COLS
C_CC = C_R + COLS
C_ONE = C_CC + COLS
CW_TOT = C_ONE + 1

F32 = mybir.dt.float32
I32 = mybir.dt.int32
AX = mybir.AxisListType.X
OP = mybir.AluOpType
AF = mybir.ActivationFunctionType

_nc_cache = None


def _build_program():
    global _nc_cache
    if _nc_cache is not None:
        return _nc_cache

    nc = bacc.Bacc(
        "TRN2", target_bir_lowering=False, debug=False, num_devices=N_CORES
    )
    emb = nc.dram_tensor("emb", [VOCAB, DIM], F32, kind="ExternalInput")
    idx_d = nc.dram_tensor("idx32", [P, COLS], I32, kind="ExternalInput")
    coef_d = nc.dram_tensor("coefs", [P, CW_TOT], F32, kind="ExternalInput")
    out_d = nc.dram_tensor("out", [1, 1], F32, kind="ExternalOutput")

    from contextlib import ExitStack

    with ExitStack() as ctx:
        block = ctx.enter_context(nc.Block())
        sb = lambda name, shape, dt=F32: ctx.enter_context(
            nc.sbuf_tensor(name, shape, dt)
        )
        idx_sb = sb("idx_sb", [P, COLS], I32)
        coef_sb = sb("coef_sb", [P, CW_TOT])
        rows = sb("rows", [P, COLS, DIM])
        s_act = sb("s_act", [P, COLS, DIM])
        s_dve = sb("s_dve", [P, COLS, DIM])
        rowsum = sb("rowsum", [P, COLS])
        tcol = sb("tcol", [P, COLS])
        a13 = sb("a13", [P, COLS])
        u13 = sb("u13", [P, COLS])
        w13 = sb("w13", [P, COLS])
        x1 = sb("x1", [P, COLS])
        x2 = sb("x2", [P, COLS])
        x3 = sb("x3", [P, COLS])
        m13 = sb("m13", [P, COLS])
        Lc = sb("Lc", [P, 1])
        res = sb("res", [1, 1])
        acc = ctx.enter_context(nc.psum_tensor([1, 1], F32))
        sem = lambda name: ctx.enter_context(nc.semaphore(name))
        io, io_i, io2 = sem("io"), sem("io_i"), sem("io2")
        gs = [sem(f"gs{j}") for j in range(COLS)]
        dve_x, act_s, dve_f = sem("dve_x"), sem("act_s"), sem("dve_f")
        pe_s, cp_s = sem("pe_s"), sem("cp_s")
        chain_len = {}
        ramp = coef_sb[:, 0:CW_RAMP]
        ones = coef_sb[:, C_ONE : C_ONE + 1]

        @block.sync
        def _(sync: bass.BassEngine):
            sync.dma_start(idx_sb[:, :], idx_d[:, :]).then_inc(io_i, 16)
            sync.dma_start(coef_sb[:, :], coef_d[:, :]).then_inc(io, 16)
            sync.wait_ge(cp_s, 1)
            sync.dma_start(out_d[:, :], res[:, :]).then_inc(io2, 16)
            sync.wait_ge(io2, 16)

        @block.gpsimd
        def _(gpsimd: bass.BassGpSimd):
            gpsimd.wait_ge(io_i, 16)
            for j in range(COLS):
                gpsimd.indirect_dma_start(
                    out=rows[:, j, :],
                    out_offset=None,
                    in_=emb[:, :],
                    in_offset=bass.IndirectOffsetOnAxis(
                        ap=idx_sb[:, j : j + 1], axis=0
                    ),
                ).then_inc(gs[j], 16)

        @block.scalar
        def _(scalar: bass.BassEngine):
            for j in range(COLS):
                scalar.wait_ge(gs[j], 16)
                nc.scalar.activation(
                    s_act[:, j, :], rows[:, j, :], AF.Abs,
                    accum_out=rowsum[:, j : j + 1],
                ).then_inc(act_s, 1)
            scalar.wait_ge(pe_s, 1)
            nc.scalar.copy(res[:, :], acc[:, :]).then_inc(cp_s, 1)

        @block.vector
        def _(vector: bass.BassEngine):
            vector.wait_ge(io, 16)
            for j in range(COLS):
                vector.wait_ge(gs[j], 16)
                nc.vector.scalar_tensor_tensor(
                    out=s_dve[:, j, :],
                    in0=ramp,
                    scalar=coef_sb[:, C_DIMS + j : C_DIMS + j + 1],
                    in1=rows[:, j, :],
                    op0=OP.is_equal,
                    op1=OP.mult,
                    accum_out=tcol[:, j : j + 1],
                ).then_inc(dve_x, 1)
            # accum_out writes land late; drain our own pipeline before reads
            vector.wait_ge(dve_x, COLS)
            # Same-engine RAW needs explicit sems (deep DVE pipeline).
            # dve_f counts completions; wait on the latest producer.
            # L = w*(Q*a + R) + Pp*a + Cc*rowsum,  w = (t*S>=0), a = |t|
            n = 0

            def step(ins, wait=None):
                nonlocal n
                if wait is not None:
                    vector.wait_ge(dve_f, wait)
                ins().then_inc(dve_f, 1)
                n += 1
                return n

            tS = coef_sb[:, C_S : C_S + COLS]
            i_u = step(lambda: nc.vector.tensor_tensor(
                out=u13[:, :], in0=tcol[:, :], in1=tS, op=OP.mult))
            i_w = step(lambda: nc.vector.tensor_scalar(
                out=w13[:, :], in0=u13[:, :], scalar1=0.0, scalar2=None,
                op0=OP.is_ge), wait=i_u)
            i_m = step(lambda: nc.vector.tensor_scalar(
                out=m13[:, :], in0=w13[:, :], scalar1=2.0, scalar2=-1.0,
                op0=OP.mult, op1=OP.add), wait=i_w)
            i_a = step(lambda: nc.vector.tensor_tensor(
                out=a13[:, :], in0=u13[:, :], in1=m13[:, :], op=OP.mult),
                wait=i_m)
            i1 = step(lambda: nc.vector.tensor_tensor(
                out=x1[:, :], in0=a13[:, :], in1=coef_sb[:, C_Q : C_Q + COLS],
                op=OP.mult), wait=i_a)
            i2 = step(lambda: nc.vector.tensor_tensor(
                out=x2[:, :], in0=a13[:, :], in1=coef_sb[:, C_PP : C_PP + COLS],
                op=OP.mult))
            vector.wait_ge(act_s, COLS)
            i3 = step(lambda: nc.vector.tensor_tensor(
                out=x3[:, :], in0=rowsum[:, :], in1=coef_sb[:, C_CC : C_CC + COLS],
                op=OP.mult))
            i4 = step(lambda: nc.vector.tensor_tensor(
                out=x1[:, :], in0=x1[:, :], in1=coef_sb[:, C_R : C_R + COLS],
                op=OP.add), wait=i1)
            i5 = step(lambda: nc.vector.tensor_tensor(
                out=x1[:, :], in0=x1[:, :], in1=w13[:, :], op=OP.mult), wait=i4)
            i6 = step(lambda: nc.vector.tensor_tensor(
                out=x1[:, :], in0=x1[:, :], in1=x2[:, :], op=OP.add),
                wait=max(i5, i2))
            i7 = step(lambda: nc.vector.tensor_tensor(
                out=x1[:, :], in0=x1[:, :], in1=x3[:, :], op=OP.add),
                wait=max(i6, i3))
            i8 = step(lambda: nc.vector.reduce_sum(
                Lc[:, :], x1[:, :], axis=AX), wait=i7)
            chain_len["n"] = i8

        @block.tensor
        def _(tensor: bass.BassEngine):
            tensor.wait_ge(io, 16)
            tensor.wait_ge(dve_f, chain_len["n"])
            nc.tensor.matmul(
                acc[:, :], lhsT=ones, rhs=Lc[:, :], start=True, stop=True
            ).then_inc(pe_s, 1)

    nc.compile()
    _nc_cache = nc
    return nc


def _deal(pos_ids, pos_dims, neg_ids, neg_dims, neu_ids, neu_dims):
    """Deal all constraints into per-core slot tables (slot j of core c =
    constraint c + 8*j of the concatenated list).

    Returns per-core (idx32 [128, COLS] int32, coefs [128, CW_TOT] f32).
    """
    ids = np.concatenate([pos_ids, neg_ids, neu_ids]).astype(np.int64)
    dims = np.concatenate([pos_dims, neg_dims, neu_dims]).astype(np.int64)
    cls = np.concatenate([
        np.zeros(len(pos_ids), np.int64),
        np.ones(len(neg_ids), np.int64),
        np.full(len(neu_ids), 2, np.int64),
    ])

    idx32 = []
    coefs = []
    for c in range(N_CORES):
        g = np.arange(SLOTS) * N_CORES + c  # this core's constraints
        cid, cdim, ccls = ids[g], dims[g], cls[g]
        # slot j -> (p = j%128, col = j//128)
        ix = np.ascontiguousarray(
            cid.reshape(COLS, P).T.astype(np.int32))  # [128, COLS]
        cf = np.zeros((P, CW_TOT), np.float32)
        cf[:, 0:CW_RAMP] = np.arange(DIM, dtype=np.float32)[None, :]
        cf[:, C_ONE] = 1.0
        dm = cdim.reshape(COLS, P).T
        kl = ccls.reshape(COLS, P).T
        cf[:, C_DIMS : C_DIMS + COLS] = dm
        cf[:, C_S : C_S + COLS] = np.where(kl == 0, -1.0, 1.0)
        pn = kl != 2
        cf[:, C_PP : C_PP + COLS] = np.where(
            pn, -SPARSITY_WEIGHT - C_SP, 2.0)
        cf[:, C_Q : C_Q + COLS] = np.where(pn, 1.0 + SPARSITY_WEIGHT, 0.0)
        cf[:, C_R : C_R + COLS] = np.where(pn, SPARSITY_WEIGHT, 0.0)
        cf[:, C_CC : C_CC + COLS] = np.where(pn, C_SP, 0.0)
        idx32.append(ix)
        coefs.append(cf)
    return idx32, coefs


def _make_in_maps(emb, pos_ids, pos_dims, neg_ids, neg_dims, neu_ids, neu_dims):
    idx32, coefs = _deal(pos_ids, pos_dims, neg_ids, neg_dims, neu_ids, neu_dims)
    return [
        {"emb": emb, "idx32": idx32[c], "coefs": coefs[c]}
        for c in range(N_CORES)
    ]


def kernel(**inputs):
    emb = np.ascontiguousarray(np.asarray(inputs["embeddings"], dtype=np.float32))
    ids = {
        k: np.asarray(inputs[k]).astype(np.int64)
        for k in ("pos_ids", "pos_dims", "neg_ids", "neg_dims", "neu_ids", "neu_dims")
    }
    nc = _build_program()
    in_maps = _make_in_maps(
        emb, ids["pos_ids"], ids["pos_dims"], ids["neg_ids"], ids["neg_dims"],
        ids["neu_ids"], ids["neu_dims"],
    )
    res = run_bass_kernel_spmd(nc, in_maps, list(range(N_CORES)))
    total = sum(float(r["out"][0, 0]) for r in res.results)
    val = total * CONSISTENCY_WEIGHT / N_ALL
    return np.asarray(val, dtype=np.float32)


# revision 20
# speedup vs baseline: 1.4247x; 1.0288x over previous
"""DimensionalConsistencyLoss on 8 Trainium2 NeuronCores.

The loss touches only gathered rows of the [100000, 512] f32 table: 8192
pos/neg constraints read one row each (sparsity term + target element), 2048
neu constraints read one element. Everything is fetched with row gathers.

Per core (1/8 of the constraints = 1280 slots = 10 columns of 128, dealt by
the host):
  - 10x indirect-DMA row gathers (one [128,512] tile per column; the SWDGE
    Q7 feeds descriptors faster than the 16 SDMA engines drain them, and
    unlike dma_gather this needs no ucode-library load, which costs ~10us).
  - ACT: per tile, activation(Abs, accum_out) -> row |.| sums in one pass.
  - DVE: per tile, scalar_tensor_tensor((ramp == dim_p) * row, accum_out)
    extracts the target element t in one pass.
  - Per-slot coefficient arrays (host-built) unify pos/neg/neu:
        L = w*(Q*|t| + R) + P*|t| + C*rowsum,   w = (S*t >= 0)
  - ones-matmul reduces the [128, COLS] loss matrix to one scalar.

Host sums 8 partial scalars and applies the final scale.
"""

import numpy as np

import concourse.bacc as bacc
import concourse.bass as bass
import concourse.mybir as mybir
from concourse.bass_utils import run_bass_kernel_spmd

P = 128
VOCAB = 100000
DIM = 512
N_POS = 4096
N_NEG = 4096
N_NEU = 2048
N_ALL = N_POS + N_NEG + N_NEU
N_CORES = 8

SLOTS = N_ALL // N_CORES           # 1280
COLS = SLOTS // P                  # 10
RCOLS = (N_POS + N_NEG) // N_CORES // P   # 8 row-gather columns (pos/neg)
# cols RCOLS..COLS-1 are neu: element gathers land t directly in tcol

CONSISTENCY_WEIGHT = 0.5
SPARSITY_WEIGHT = 0.1
C_SP = SPARSITY_WEIGHT / (DIM - 1)

# coefs tensor layout (f32, [128, CW_TOT]): ramp | dims | S | Pp | Q | R | Cc | ones
CW_RAMP = DIM
C_DIMS = CW_RAMP
C_S = C_DIMS + COLS
C_PP = C_S + COLS
C_Q = C_PP + COLS
C_R = C_Q + COLS
C_CC = C_R + COLS
C_ONE = C_CC + COLS
CW_TOT = C_ONE + 1

F32 = mybir.dt.float32
I32 = mybir.dt.int32
AX = mybir.AxisListType.X
OP = mybir.AluOpType
AF = mybir.ActivationFunctionType

_nc_cache = None


def _build_program():
    global _nc_cache
    if _nc_cache is not None:
        return _nc_cache

    nc = bacc.Bacc(
        "TRN2", target_bir_lowering=False, debug=False, num_devices=N_CORES
    )
    emb = nc.dram_tensor("emb", [VOCAB, DIM], F32, kind="ExternalInput")
    idx_d = nc.dram_tensor("idx32", [P, COLS], I32, kind="ExternalInput")
    coef_d = nc.dram_tensor("coefs", [P, CW_TOT], F32, kind="ExternalInput")
    out_d = nc.dram_tensor("out", [P, COLS], F32, kind="ExternalOutput")

    from contextlib import ExitStack

    with ExitStack() as ctx:
        block = ctx.enter_context(nc.Block())
        sb = lambda name, shape, dt=F32: ctx.enter_context(
            nc.sbuf_tensor(name, shape, dt)
        )
        idx_sb = sb("idx_sb", [P, COLS], I32)
        coef_sb = sb("coef_sb", [P, CW_TOT])
        rows = sb("rows", [P, RCOLS, DIM])
        s_act = sb("s_act", [P, RCOLS, DIM])
        s_dve = sb("s_dve", [P, RCOLS, DIM])
        rowsum = sb("rowsum", [P, COLS])
        tcol = sb("tcol", [P, COLS])
        a13 = sb("a13", [P, COLS])
        u13 = sb("u13", [P, COLS])
        w13 = sb("w13", [P, COLS])
        x1 = sb("x1", [P, COLS])
        x2 = sb("x2", [P, COLS])
        x3 = sb("x3", [P, COLS])
        m13 = sb("m13", [P, COLS])
        sem = lambda name: ctx.enter_context(nc.semaphore(name))
        io, io_i, io2 = sem("io"), sem("io_i"), sem("io2")
        gs = [sem(f"gs{j}") for j in range(COLS)]
        dve_x, act_s, dve_f = sem("dve_x"), sem("act_s"), sem("dve_f")
        chain_len = {}
        ramp = coef_sb[:, 0:CW_RAMP]

        @block.gpsimd
        def _(gpsimd: bass.BassGpSimd):
            gpsimd.dma_start(idx_sb[:, :], idx_d[:, :]).then_inc(io_i, 16)
            gpsimd.wait_ge(io_i, 16)
            for j in range(RCOLS):
                gpsimd.indirect_dma_start(
                    out=rows[:, j, :],
                    out_offset=None,
                    in_=emb[:, :],
                    in_offset=bass.IndirectOffsetOnAxis(
                        ap=idx_sb[:, j : j + 1], axis=0
                    ),
                ).then_inc(gs[j], 16)
            for j in range(RCOLS, COLS):
                # neu: flat element gather (idx = id*DIM+dim) lands t directly
                gpsimd.indirect_dma_start(
                    out=tcol[:, j : j + 1],
                    out_offset=None,
                    in_=emb[:, :],
                    in_offset=bass.IndirectOffsetOnAxis(
                        ap=idx_sb[:, j : j + 1], axis=1
                    ),
                ).then_inc(gs[j], 16)

        @block.scalar
        def _(scalar: bass.BassEngine):
            for j in range(RCOLS):
                scalar.wait_ge(gs[j], 16)
                nc.scalar.activation(
                    s_act[:, j, :], rows[:, j, :], AF.Abs,
                    accum_out=rowsum[:, j : j + 1],
                ).then_inc(act_s, 1)

        @block.vector
        def _(vector: bass.BassEngine):
            vector.wait_ge(io, 16)
            for j in range(RCOLS):
                vector.wait_ge(gs[j], 16)
                nc.vector.scalar_tensor_tensor(
                    out=s_dve[:, j, :],
                    in0=ramp,
                    scalar=coef_sb[:, C_DIMS + j : C_DIMS + j + 1],
                    in1=rows[:, j, :],
                    op0=OP.is_equal,
                    op1=OP.mult,
                    accum_out=tcol[:, j : j + 1],
                ).then_inc(dve_x, 1)
            # accum_out writes land late; drain our own pipeline before reads
            vector.wait_ge(dve_x, RCOLS)
            for j in range(RCOLS, COLS):
                vector.wait_ge(gs[j], 16)
            # Same-engine RAW needs explicit sems (deep DVE pipeline).
            # dve_f counts completions; wait on the latest producer.
            # L = w*(Q*a + R) + Pp*a + Cc*rowsum,  w = (t*S>=0), a = |t|
            n = 0

            def step(ins, wait=None):
                nonlocal n
                if wait is not None:
                    vector.wait_ge(dve_f, wait)
                ins().then_inc(dve_f, 1)
                n += 1
                return n

            tS = coef_sb[:, C_S : C_S + COLS]
            i_u = step(lambda: nc.vector.tensor_tensor(
                out=u13[:, :], in0=tcol[:, :], in1=tS, op=OP.mult))
            i_w = step(lambda: nc.vector.tensor_scalar(
                out=w13[:, :], in0=u13[:, :], scalar1=0.0, scalar2=None,
                op0=OP.is_ge), wait=i_u)
            i_m = step(lambda: nc.vector.tensor_scalar(
                out=m13[:, :], in0=w13[:, :], scalar1=2.0, scalar2=-1.0,
                op0=OP.mult, op1=OP.add), wait=i_w)
            i_a = step(lambda: nc.vector.tensor_tensor(
                out=a13[:, :], in0=u13[:, :], in1=m13[:, :], op=OP.mult),
                wait=i_m)
            i1 = step(lambda: nc.vector.tensor_tensor(
                out=x1[:, :], in0=a13[:, :], in1=coef_sb[:, C_Q : C_Q + COLS],
                op=OP.mult), wait=i_a)
            i2 = step(lambda: nc.vector.tensor_tensor(
                out=x2[:, :], in0=a13[:, :], in1=coef_sb[:, C_PP : C_PP + COLS],
                op=OP.mult))
            vector.wait_ge(act_s, RCOLS)
            i3 = step(lambda: nc.vector.tensor_tensor(
                out=x3[:, 0:RCOLS], in0=rowsum[:, 0:RCOLS],
                in1=coef_sb[:, C_CC : C_CC + RCOLS], op=OP.mult))
            i4 = step(lambda: nc.vector.tensor_tensor(
                out=x1[:, :], in0=x1[:, :], in1=coef_sb[:, C_R : C_R + COLS],
                op=OP.add), wait=i1)
            i5 = step(lambda: nc.vector.tensor_tensor(
                out=x1[:, :], in0=x1[:, :], in1=w13[:, :], op=OP.mult), wait=i4)
            i6 = step(lambda: nc.vector.tensor_tensor(
                out=x1[:, :], in0=x1[:, :], in1=x2[:, :], op=OP.add),
                wait=max(i5, i2))
            i7 = step(lambda: nc.vector.tensor_tensor(
                out=x1[:, 0:RCOLS], in0=x1[:, 0:RCOLS], in1=x3[:, 0:RCOLS],
                op=OP.add), wait=max(i6, i3))
            chain_len["n"] = i7

        @block.sync
        def _(sync: bass.BassEngine):
            sync.dma_start(coef_sb[:, :], coef_d[:, :]).then_inc(io, 16)
            sync.wait_ge(dve_f, chain_len["n"])
            sync.dma_start(out_d[:, :], x1[:, :]).then_inc(io2, 16)
            sync.wait_ge(io2, 16)


    nc.compile()
    _nc_cache = nc
    return nc


def _deal(pos_ids, pos_dims, neg_ids, neg_dims, neu_ids, neu_dims):
    """Deal all constraints into per-core slot tables (slot j of core c =
    constraint c + 8*j of the concatenated list).

    Returns per-core (idx32 [128, COLS] int32, coefs [128, CW_TOT] f32).
    """
    ids = np.concatenate([pos_ids, neg_ids, neu_ids]).astype(np.int64)
    dims = np.concatenate([pos_dims, neg_dims, neu_dims]).astype(np.int64)
    cls = np.concatenate([
        np.zeros(len(pos_ids), np.int64),
        np.ones(len(neg_ids), np.int64),
        np.full(len(neu_ids), 2, np.int64),
    ])

    idx32 = []
    coefs = []
    for c in range(N_CORES):
        g = np.arange(SLOTS) * N_CORES + c  # this core's constraints
        cid, cdim, ccls = ids[g].copy(), dims[g], cls[g]
        # neu slots gather the element directly: flat index id*DIM+dim
        cid[ccls == 2] = cid[ccls == 2] * DIM + cdim[ccls == 2]
        # slot j -> (p = j%128, col = j//128)
        ix = np.ascontiguousarray(
            cid.reshape(COLS, P).T.astype(np.int32))  # [128, COLS]
        cf = np.zeros((P, CW_TOT), np.float32)
        cf[:, 0:CW_RAMP] = np.arange(DIM, dtype=np.float32)[None, :]
        cf[:, C_ONE] = 1.0
        dm = cdim.reshape(COLS, P).T
        kl = ccls.reshape(COLS, P).T
        cf[:, C_DIMS : C_DIMS + COLS] = dm
        cf[:, C_S : C_S + COLS] = np.where(kl == 0, -1.0, 1.0)
        pn = kl != 2
        cf[:, C_PP : C_PP + COLS] = np.where(
            pn, -SPARSITY_WEIGHT - C_SP, 2.0)
        cf[:, C_Q : C_Q + COLS] = np.where(pn, 1.0 + SPARSITY_WEIGHT, 0.0)
        cf[:, C_R : C_R + COLS] = np.where(pn, SPARSITY_WEIGHT, 0.0)
        cf[:, C_CC : C_CC + COLS] = np.where(pn, C_SP, 0.0)
        idx32.append(ix)
        coefs.append(cf)
    return idx32, coefs


def _make_in_maps(emb, pos_ids, pos_dims, neg_ids, neg_dims, neu_ids, neu_dims):
    idx32, coefs = _deal(pos_ids, pos_dims, neg_ids, neg_dims, neu_ids, neu_dims)
    return [
        {"emb": emb, "idx32": idx32[c], "coefs": coefs[c]}
        for c in range(N_CORES)
    ]


def kernel(**inputs):
    emb = np.ascontiguousarray(np.asarray(inputs["embeddings"], dtype=np.float32))
    ids = {
        k: np.asarray(inputs[k]).astype(np.int64)
        for k in ("pos_ids", "pos_dims", "neg_ids", "neg_dims", "neu_ids", "neu_dims")
    }
    nc = _build_program()
    in_maps = _make_in_maps(
        emb, ids["pos_ids"], ids["pos_dims"], ids["neg_ids"], ids["neg_dims"],
        ids["neu_ids"], ids["neu_dims"],
    )
    res = run_bass_kernel_spmd(nc, in_maps, list(range(N_CORES)))
    total = sum(float(r["out"].astype(np.float64).sum()) for r in res.results)
    val = total * CONSISTENCY_WEIGHT / N_ALL
    return np.asarray(val, dtype=np.float32)
